# revision 1
# baseline (speedup 1.0000x reference)
"""AMICO ADMM solver on 8 TRN2 NeuronCores.

min_x ||y - A x||^2 + lambda*|x|_1, x >= 0 via ADMM (100 iterations),
data-parallel over voxels (1024 voxels per core).

Algebraic restructuring (rho=1, kappa=lambda/rho):
  Reference per-iteration:
    rhs = AtY + (z - u); x = W @ rhs; v = x + u
    z' = relu(v - kappa); u' = min(v, kappa)
  With s' := (z - u) + kappa = |v - kappa|, m := u = min(v, kappa),
  B := W @ AtY2 (constant, AtY2 = AtY + kappa*(AtA @ 1)), D := B - kappa:
    psum = W @ s' + D             # 8 fp16 matmuls + 4 identity-inject matmuls
                                  # (psum == x' - kappa; D injected via I @ D)
    v    = psum + m               # VectorE tensor_tensor (PSUM + SBUF)
    s'   = |v - kappa|            # ScalarE Abs activation -> fp16
    m    = min(v, kappa)          # VectorE tensor_scalar (fp16 4x perf mode)
  Final output: x_100 = psum_100 directly.

Precision: the PSUM accumulation is fp32 and the constant B is computed ONCE
on device as (W A^T) @ Y in fp32r (a fixed ~1e-3-class bias on the constant)
with the (W c* - kappa) vector folded in as a per-partition bias (P = W A^T
and w2 = W c* are tiny host-side f64 precomputes). The fp16 (10-bit mantissa ~ fp32r's 11) quantization of
W/s'/v/m then only perturbs the small per-iteration residual, not the large
constant term: measured rel_l2 4.6e-3 (vs 1.1e-2 when W @ AtY2 itself ran
in reduced precision). fp16 weights get fast-weight-load, roughly halving
the per-matmul weight-reload cost vs fp32r.

All elementwise work is chunked per 512 columns with separate tiles so that
iteration i+1's matmuls can start as soon as the first column chunk of s'
is ready; the identity-inject matmuls are emitted first in each iteration
(no s' dependency), giving the PE runway while the v/abs chain completes.
"""

import os

import numpy as np

M = 256
K = 256
N_VOX = 8192
N_CORES = 8
N_SHARD = N_VOX // N_CORES  # 1024
RHO = 1.0
LAMBDA_REG = 0.1
KAPPA = LAMBDA_REG / RHO
N_ITERS = 100

LAST_RESULTS = None  # BassKernelResults of the most recent run (for test.py)


def _build_graph():
    import concourse.mybir as mybir
    from concourse import bacc
    from concourse.tile import TileContext

    f32 = mybir.dt.float32
    f32r = mybir.dt.float32r
    fp16 = mybir.dt.float16
    kap = float(KAPPA)

    nc = bacc.Bacc("TRN2", target_bir_lowering=False, debug=False)

    # Y[mc*128+p, n]    at Y_p[p, mc*1024 + n]      (Y = data_shard.T)
    Y_p = nc.declare_dram_parameter("Y", [128, 2048], f32r, isOutput=False)
    # P = W @ A.T (host);  P.T[mc*128+p, k] at P_p[p, mc*256 + k]
    P_p = nc.declare_dram_parameter("Pmat", [128, 512], f32r, isOutput=False)
    # w2[k] = (W @ cstar)[k] - kappa,  cstar = kappa * (AtA @ ones);
    # stored per r-block: w2_p[p, r] = w2[r*128+p] - kappa
    C_p = nc.declare_dram_parameter("w2", [128, 2], f32, isOutput=False)
    # 128x128 identity (fp16) for the D-injection matmuls
    I_p = nc.declare_dram_parameter("ident", [128, 128], fp16, isOutput=False)
    # W again in fp16 for the per-iteration matmuls
    W16_p = nc.declare_dram_parameter("W16", [128, 512], fp16, isOutput=False)
    # x[r*128+p, n]     at O_p[p, r*1024 + n]
    O_p = nc.declare_dram_parameter("out", [128, 2048], f32, isOutput=True)

    absf = mybir.ActivationFunctionType.Abs

    with TileContext(nc) as tc:
        with (
            tc.tile_pool(name="static", bufs=1) as statics,
            tc.tile_pool(name="spool", bufs=8) as spool,
            tc.tile_pool(name="vpool", bufs=8) as vpool,
            tc.tile_pool(name="mpool", bufs=8) as mpool,
        ):
            Y_sb = statics.tile([128, 2048], f32r, name="Y_sb")
            nc.sync.dma_start(Y_sb[:, :], Y_p[:, :])
            P_sb = statics.tile([128, 512], f32r, name="P_sb")
            nc.sync.dma_start(P_sb[:, :], P_p[:, :])
            c_sb = statics.tile([128, 2], f32, name="c_sb")
            nc.sync.dma_start(c_sb[:, :], C_p[:, :])
            i_sb = statics.tile([128, 128], fp16, name="i_sb")
            nc.sync.dma_start(i_sb[:, :], I_p[:, :])
            W16_sb = statics.tile([128, 512], fp16, name="W16_sb")
            nc.sync.dma_start(W16_sb[:, :], W16_p[:, :])
            out_sb = statics.tile([128, 2048], f32, name="out_sb")
            nkapb_sb = statics.tile([128, 1], f32, name="nkapb_sb")
            nc.vector.memset(nkapb_sb[:, :], -kap)
            kconst = statics.tile([128, 512], f32, name="kconst")
            nc.vector.memset(kconst[:, :], kap)

            # Tiny dummy Abs up front so the ~2.7us ACT_TABLE_LOAD overlaps
            # the input DMAs instead of stalling iteration 1's first real Abs.
            warm_sb = statics.tile([1, 8], f32, name="warm_sb")
            nc.scalar.activation(
                warm_sb[:, :], nkapb_sb[:1, :].to_broadcast((1, 8)), absf,
                bias=nkapb_sb[:1, :], scale=1.0,
            )

            dconst = []
            with tc.tile_pool(name="psum_setup", bufs=2, space="PSUM") as pss:
                # ---- B0 = P @ Y (fp32r, 1 cyc/row);  D = B0 + (w2 - kappa) ----
                # (w2 - kappa) enters as a per-partition scalar in the TS.
                for r in (0, 1):
                    ps = pss.tile([128, 1024], f32, name="ps_b", tag="pss")
                    for c in (0, 1):
                        dst = ps[:, c * 512 : (c + 1) * 512]
                        for mc in (0, 1):
                            nc.tensor.matmul(
                                dst,
                                P_sb[:, mc * 256 + r * 128 : mc * 256 + r * 128 + 128],
                                Y_sb[
                                    :, mc * 1024 + c * 512 : mc * 1024 + c * 512 + 512
                                ],
                                start=(mc == 0),
                                stop=(mc == 1),
                            )
                    d_r = statics.tile([128, 1024], fp16, name=f"dconst_{r}")
                    nc.vector.tensor_scalar_add(
                        d_r[:, :], ps[:, :], c_sb[:, r : r + 1]
                    )
                    dconst.append(d_r)

            # ---- init: s'_0 = kappa (fp16), m_0 = 0 ----
            s_h = [[None, None], [None, None]]
            m_h = [[None, None], [None, None]]
            for h in (0, 1):
                for c in (0, 1):
                    s0 = spool.tile([128, 512], fp16, name="s_new", tag="s")
                    nc.vector.tensor_copy(s0[:, :], kconst[:, :])
                    s_h[h][c] = s0
                    m0 = mpool.tile([128, 512], fp16, name="m_new", tag="m")
                    nc.vector.memset(m0[:, :], 0.0)
                    m_h[h][c] = m0

            # ---- 100 ADMM iterations, fully unrolled ----
            with tc.tile_pool(name="psum_loop", bufs=8, space="PSUM") as psl:
                for it in range(N_ITERS):
                    last = it == N_ITERS - 1
                    ps_rc = [[None, None], [None, None]]
                    # All four D-injections first: they have no s' dependency,
                    # giving the PE ~1.2us of runway while the previous
                    # iteration's v/abs chain produces s'. W-groups follow in
                    # c-major order so c=0's PSUM completes early.
                    for c in (0, 1):
                        for r in (0, 1):
                            ps = psl.tile([128, 512], f32, name="ps_x", tag="ps")
                            nc.tensor.matmul(
                                ps[:, :],
                                i_sb[:, :],
                                dconst[r][:, c * 512 : (c + 1) * 512],
                                start=True,
                                stop=False,
                                skip_group_check=True,
                            )
                            ps_rc[r][c] = ps
                    for c in (0, 1):
                        for r in (0, 1):
                            for kc in (0, 1):
                                w0 = kc * 256 + r * 128
                                nc.tensor.matmul(
                                    ps_rc[r][c][:, :],
                                    W16_sb[:, w0 : w0 + 128],
                                    s_h[kc][c][:, :],
                                    start=False,
                                    stop=(kc == 1),
                                    skip_group_check=True,
                                )

                    if last:
                        for h in (0, 1):
                            for c in (0, 1):
                                dst = out_sb[
                                    :, h * 1024 + c * 512 : h * 1024 + c * 512 + 512
                                ]
                                nc.scalar.copy(dst, ps_rc[h][c][:, :])
                        break

                    new_s = [[None, None], [None, None]]
                    new_m = [[None, None], [None, None]]
                    for c in (0, 1):
                        vs = []
                        for h in (0, 1):
                            v = vpool.tile([128, 512], fp16, name="v", tag="v")
                            nc.vector.tensor_add(
                                v[:, :], ps_rc[h][c][:, :], m_h[h][c][:, :]
                            )
                            sn = spool.tile([128, 512], fp16, name="s_new", tag="s")
                            nc.scalar.activation(
                                sn[:, :], v[:, :], absf, bias=nkapb_sb[:, :], scale=1.0
                            )
                            new_s[h][c] = sn
                            vs.append(v)
                        for h in (0, 1):
                            mn = mpool.tile([128, 512], fp16, name="m_new", tag="m")
                            nc.vector.tensor_scalar_min(mn[:, :], vs[h][:, :], kap)
                            new_m[h][c] = mn
                    s_h, m_h = new_s, new_m


            nc.sync.dma_start(O_p[:, :], out_sb[:, :])

    nc.compile()
    return nc


_GRAPH = None


def kernel(A: np.ndarray, data: np.ndarray) -> np.ndarray:
    global _GRAPH, LAST_RESULTS
    from concourse.bass_utils import run_bass_kernel_spmd

    A = np.ascontiguousarray(np.asarray(A, dtype=np.float32))
    data = np.ascontiguousarray(np.asarray(data, dtype=np.float32))
    assert A.shape == (M, K) and data.shape == (N_VOX, M)

    # Host-side tiny precompute (all [K,K]-sized, f64):
    #   W = (AtA + rho I)^-1 (symmetric), P = W @ A.T, w2 = W @ cstar.
    A64 = A.astype(np.float64)
    AtA = A64.T @ A64
    W = np.linalg.inv(AtA + RHO * np.eye(K))
    cstar = KAPPA * (AtA @ np.ones(K))
    PT = A64 @ W  # = (W @ A.T).T since W is symmetric
    w2 = W @ cstar

    # Device layouts.
    P_dev = (
        PT.astype(np.float32).reshape(2, 128, K).transpose(1, 0, 2).reshape(128, 2 * K)
    )
    W_dev = (
        W.astype(np.float32).reshape(2, 128, K).transpose(1, 0, 2).reshape(128, 2 * K)
    )
    c_dev = (
        (w2 - KAPPA).astype(np.float32).reshape(2, 128).T
    )  # [128, 2] per r-block
    i_dev = np.eye(128, dtype=np.float16)
    W16_dev = W_dev.astype(np.float16)

    in_maps = []
    for i in range(N_CORES):
        shard = data[i * N_SHARD : (i + 1) * N_SHARD]  # [1024, 256]
        Yt = np.ascontiguousarray(shard.T)  # [256, 1024]
        Y_dev = Yt.reshape(2, 128, N_SHARD).transpose(1, 0, 2).reshape(128, 2 * N_SHARD)
        in_maps.append(
            {
                "Y": np.ascontiguousarray(Y_dev),
                "Pmat": np.ascontiguousarray(P_dev),
                "w2": np.ascontiguousarray(c_dev),
                "ident": i_dev,
                "W16": W16_dev,
            }
        )

    if _GRAPH is None:
        _GRAPH = _build_graph()

    trace = bool(int(os.environ.get("KERNEL_TRACE", "0")))
    res = run_bass_kernel_spmd(
        _GRAPH, in_maps, core_ids=list(range(N_CORES)), trace=trace
    )
    LAST_RESULTS = res

    out = np.empty((N_VOX, K), dtype=np.float32)
    for i in range(N_CORES):
        o = res.results[i]["out"]  # [128, 2048]
        X = o.reshape(128, 2, N_SHARD).transpose(1, 0, 2).reshape(K, N_SHARD)
        out[i * N_SHARD : (i + 1) * N_SHARD] = X.T
    return out

